# revision 28
# baseline (speedup 1.0000x reference)
"""Trainium2 Bass kernel: batched dense attention
   out = softmax((x_q Wq^T + bq)(x_k Wk^T + bk)^T / sqrt(E)) (x_v Wv^T + bv)

Sharding: 8 cores = 4 batches x 2 query-row halves (sequence-parallel on
Q). K/V projections are also split across the pair (each core projects
its own half of K/V) and the halves are exchanged with chunked in-pair
AllGathers that pipeline behind the projection matmuls, so every core
does exactly 1/8 of the total FLOPs.

Key trick: softmax attention is invariant to a permutation of the key
axis, so each core uses its own LOCAL key order [my half; partner half].
The locally projected half is written straight into the resident SBUF
K^T/V tiles (no DRAM round trip), and only the partner block of each
AllGather output is loaded back, addressed with a partition_id-derived
dynamic DRAM offset.

Device scheme (matmul operands bf16, fp32 PSUM accumulation):
  - scores are computed TRANSPOSED (keys on partitions) so the exp'd
    probabilities feed the P@V matmul as the stationary operand with no
    on-device transposes (host pre-transposes x^T / W^T and pre-casts
    to bf16).
  - softmax without max-subtraction (scores ~ N(0,1) at this scale;
    exp is safe in fp32): Z = sum_k exp(s) accumulated via a
    ones-vector matmul; 1/Z applied during the PSUM->SBUF output copy.
  - attention consumes key-tiles in availability order (own tiles
    first within each gather chunk), and the P@V matmuls trail the
    score/exp pipeline by one key-tile so the PSUM output-slot handoff
    between query chunks stays off the TensorE critical path.
  - bv folded to the end (softmax rows sum to 1 => P@(V+bv) = P@V+bv),
    and skipped entirely when bv == 0 (a per-bias-pattern kernel
    variant is compiled).
"""

import numpy as np
import ml_dtypes

import concourse.bacc as bacc
import concourse.bass as bass
import concourse.mybir as mybir
import concourse.tile as tile
from concourse.bass_utils import run_bass_kernel_spmd

B, S, E = 4, 4096, 1024
N_CORES = 8
HQ = 2              # halves per batch
SQ = S // HQ        # 2048 rows per core (own query rows / own K,V rows)
P = 128
ET = E // P         # 8 embed tiles
SKT = S // P        # 32 key tiles (local order: 0-15 own, 16-31 partner)
NQ = 256            # attention query-chunk (2 psum out subtiles)
NQT = SQ // NQ      # 8 chunks per core
CH = 512            # projection column chunk
HCH = SQ // CH      # 4 chunks for the K/V half projections
INV_SCALE = 1.0 / float(E) ** 0.5
GROUPS = [[0, 1], [2, 3], [4, 5], [6, 7]]

# key-tile processing order: chunk ch covers own tiles {4ch..4ch+3}
# (ready right after the projection) and partner tiles {16+4ch..}
# (ready after that chunk's AllGather)
SK_ORDER = list(range(16)) + list(range(16, 32))

BF16 = mybir.dt.bfloat16
F32 = mybir.dt.float32
AF = mybir.ActivationFunctionType

_CACHE = {}


def _emit(nc, tc, dram, with_bv):
    xqT, xkT, xvT, wqT, wkT, wvT, bqr, bkr, bvb, out = dram

    # partner-block row offsets into the AllGather outputs (per-engine
    # registers: dynamic DMA offsets must live on the issuing engine)
    pid_g = nc.gpsimd.partition_id()
    off_k = (1 - (pid_g & 1)) * E
    off_v = (1 - (pid_g & 1)) * CH

    with (
        tc.tile_pool(name="consts", bufs=1) as cpool,
        tc.tile_pool(name="kv", bufs=1) as kvpool,
        tc.tile_pool(name="ccd", bufs=1, space="DRAM") as dpool,
    ):
        bq_sb = cpool.tile([P, ET], F32)
        nc.gpsimd.dma_start(bq_sb[:], bqr[:])
        bk_sb = cpool.tile([P, ET], F32)
        nc.gpsimd.dma_start(bk_sb[:], bkr[:])
        if with_bv:
            bv_sb = cpool.tile([P, E], F32)
            nc.gpsimd.dma_start(bv_sb[:], bvb[:])

        # tiny warm-up collective: pays the first-collective comm setup
        # (~25us) before the real exchange needs the CC cores
        warm_in = dpool.tile([1, ET], F32, tag="warm_i", name="warm_i")
        warm_out = dpool.tile([HQ, ET], F32, tag="warm_o", name="warm_o")
        nc.gpsimd.dma_start(warm_in[:], bqr[0:1, :])
        nc.gpsimd.collective_compute(
            "AllGather", mybir.AluOpType.bypass, replica_groups=GROUPS,
            ins=[warm_in.opt()], outs=[warm_out.opt()])

        ones = cpool.tile([P, 1], F32)
        nc.gpsimd.memset(ones[:], 1.0)

        wqpool = tc.alloc_tile_pool(name="wq", bufs=1)
        wq_sb = [wqpool.tile([P, E], BF16, tag=f"wq{dt}", name=f"wq{dt}")
                 for dt in range(ET)]

        # resident full K^T [E, S] and V [S, E] (bf16, 16 MB), local key order
        kT = [kvpool.tile([P, S], BF16, tag=f"kT{et}", name=f"kT{et}")
              for et in range(ET)]
        vN = [kvpool.tile([P, E], BF16, tag=f"v{sk}", name=f"v{sk}")
              for sk in range(SKT)]

        # per-chunk AllGather bounce buffers
        kb_ib = [dpool.tile([E, CH], BF16, tag=f"kbi{c}", name=f"kbi{c}")
                 for c in range(HCH)]
        kb_ob = [dpool.tile([HQ * E, CH], BF16, tag=f"kbo{c}", name=f"kbo{c}")
                 for c in range(HCH)]
        vb_ib = [dpool.tile([CH, E], BF16, tag=f"vbi{c}", name=f"vbi{c}")
                 for c in range(HCH)]
        vb_ob = [dpool.tile([HQ * CH, E], BF16, tag=f"vbo{c}", name=f"vbo{c}")
                 for c in range(HCH)]

        # ---- Phase 1: half K / half V projections + pipelined AllGathers ----
        qxpool = tc.alloc_tile_pool(name="qx", bufs=8)
        xq0 = [qxpool.tile([P, NQ], BF16, tag="qx", name="xq0")
               for _ in range(ET)]
        with (
            tc.tile_pool(name="wkv", bufs=16) as wpool,
            tc.tile_pool(name="xk", bufs=14) as xkpool,
            tc.tile_pool(name="xv", bufs=13) as xvpool,
            tc.tile_pool(name="psk", bufs=4, space="PSUM") as psk,
            tc.tile_pool(name="psv", bufs=4, space="PSUM") as psv,
        ):
            wk_sb = [wpool.tile([P, E], BF16, tag="w", name="wk")
                     for _ in range(ET)]
            for dt in range(ET):
                nc.scalar.dma_start(wk_sb[dt][:], wkT[dt * P:(dt + 1) * P, :])

            def feed(src_t, ch, pool):
                cs = slice(ch * CH, (ch + 1) * CH)
                ts = [pool.tile([P, CH], BF16, tag="x", name="xf")
                      for _ in range(ET)]
                for dt in range(ET):
                    nc.sync.dma_start(ts[dt][:], src_t[dt * P:(dt + 1) * P, cs])
                return ts

            feeds = {("k", 0): feed(xkT, 0, xkpool),
                     ("v", 0): feed(xvT, 0, xvpool)}
            wv_sb = [wpool.tile([P, E], BF16, tag="w", name="wv")
                     for _ in range(ET)]
            for dt in range(ET):
                nc.scalar.dma_start(wv_sb[dt][:], wvT[dt * P:(dt + 1) * P, :])
            for dt in range(ET):
                nc.scalar.dma_start(wq_sb[dt][:], wqT[dt * P:(dt + 1) * P, :])
            for dt in range(ET):
                nc.sync.dma_start(xq0[dt][:], xqT[dt * P:(dt + 1) * P, 0:NQ])
            for ch in range(HCH):
                cs = slice(ch * CH, (ch + 1) * CH)
                # own-half k^T[e, s-chunk] = sum_d WkT[d, e] * xkT[d, s-chunk]
                # written straight into kT columns [0, SQ)
                xs = feeds.pop(("k", ch))
                if ch + 1 < HCH:
                    feeds[("k", ch + 1)] = feed(xkT, ch + 1, xkpool)
                for et in range(ET):
                    ps = psk.tile([P, CH], F32, tag="pk")
                    for dt in range(ET):
                        nc.tensor.matmul(
                            ps[:], wk_sb[dt][:, et * P:(et + 1) * P], xs[dt][:],
                            start=(dt == 0), stop=(dt == ET - 1))
                    nc.vector.tensor_scalar_add(
                        kT[et][:, cs], ps[:], bk_sb[:, et:et + 1])
                    nc.gpsimd.dma_start(kb_ib[ch][et * P:(et + 1) * P, :],
                                        kT[et][:, cs])
                nc.gpsimd.collective_compute(
                    "AllGather", mybir.AluOpType.bypass, replica_groups=GROUPS,
                    ins=[kb_ib[ch].opt()], outs=[kb_ob[ch].opt()])

                # own-half v[s-chunk, e] = sum_d xvT[d, s-chunk] * WvT[d, e]
                xv = feeds.pop(("v", ch))
                if ch + 1 < HCH:
                    feeds[("v", ch + 1)] = feed(xvT, ch + 1, xvpool)
                for si in range(CH // P):
                    sk = ch * (CH // P) + si
                    ph = [psv.tile([P, 512], F32, tag="pv", name="pv")
                          for _ in range(2)]
                    for dt in range(ET):
                        for nh in range(2):
                            nc.tensor.matmul(
                                ph[nh][:],
                                xv[dt][:, si * P:(si + 1) * P],
                                wv_sb[dt][:, nh * 512:(nh + 1) * 512],
                                start=(dt == 0), stop=(dt == ET - 1))
                    for nh in range(2):
                        nc.vector.tensor_copy(
                            vN[sk][:, nh * 512:(nh + 1) * 512], ph[nh][:])
                    nc.gpsimd.dma_start(vb_ib[ch][si * P:(si + 1) * P, :],
                                        vN[sk][:])
                nc.gpsimd.collective_compute(
                    "AllGather", mybir.AluOpType.bypass, replica_groups=GROUPS,
                    ins=[vb_ib[ch].opt()], outs=[vb_ob[ch].opt()])

            # partner blocks -> kT columns [SQ, S) / vN tiles 16..31, deferred
            # past the projection phase: DRAM there is bandwidth-saturated by
            # feeds + bounce traffic, while attention needs these tiles only
            # ~100us later (key tiles 16+ of the first query chunk)
            for ch in range(HCH):
                for et in range(ET):
                    nc.gpsimd.dma_start(
                        kT[et][:, SQ + ch * CH:SQ + (ch + 1) * CH],
                        kb_ob[ch][bass.ds(off_k + et * P, P), :])
                for si in range(CH // P):
                    nc.gpsimd.dma_start(
                        vN[16 + ch * (CH // P) + si][:],
                        vb_ob[ch][bass.ds(off_v + si * P, P), :])

        # ---------------- Phase 2: attention ----------------
        with (
            tc.tile_pool(name="qt", bufs=16) as qtpool,
            tc.tile_pool(name="ep", bufs=4) as eppool,
            tc.tile_pool(name="acc", bufs=2) as accpool,
            tc.tile_pool(name="fin", bufs=4) as finpool,
            tc.tile_pool(name="pss", bufs=2, space="PSUM") as pss,
            tc.tile_pool(name="pso", bufs=2, space="PSUM") as pso,
            tc.tile_pool(name="psz", bufs=2, space="PSUM") as psz,
        ):
            for qc in range(NQT):
                # q^T[e, chunk] = sum_d WqT[d, e] * xqT[d, chunk]  (+ bq)
                # computed just-in-time so attention starts while the K/V
                # exchange tail is still in flight
                if qc == 0:
                    xq = xq0
                else:
                    xq = [qxpool.tile([P, NQ], BF16, tag="qx", name="xq")
                          for _ in range(ET)]
                    for dt in range(ET):
                        nc.sync.dma_start(
                            xq[dt][:],
                            xqT[dt * P:(dt + 1) * P, qc * NQ:(qc + 1) * NQ])
                qt = [qtpool.tile([P, NQ], BF16, tag="qt", name="qt")
                      for et in range(ET)]
                for et in range(ET):
                    ps = pss.tile([P, NQ], F32, tag="ps")
                    for dt in range(ET):
                        nc.tensor.matmul(
                            ps[:], wq_sb[dt][:, et * P:(et + 1) * P], xq[dt][:],
                            start=(dt == 0), stop=(dt == ET - 1))
                    nc.vector.tensor_scalar_add(qt[et][:], ps[:], bq_sb[:, et:et + 1])
                po = [pso.tile([P, E], F32, tag="po", name="po")
                      for _ in range(2)]

                def emit_pv(ep, sk, first, last):
                    for j in range(2):
                        lhs = ep[:, j * P:(j + 1) * P]
                        for nh in range(2):
                            nc.tensor.matmul(
                                po[j][:, nh * 512:(nh + 1) * 512], lhs,
                                vN[sk][:, nh * 512:(nh + 1) * 512],
                                start=first, stop=last)

                # scores/exp run one key-tile ahead of the P@V accumulation.
                # VectorE accumulates the exp'd tiles (acc = sum_sk ep) so the
                # softmax denominator needs only 2 one-column matmuls per qc
                # (partition-reduce of acc) instead of one per (key-tile, j).
                acc = accpool.tile([P, NQ], F32, tag="acc", name="acc")
                prev = None
                for idx, sk in enumerate(SK_ORDER):
                    ps = pss.tile([P, NQ], F32, tag="ps")
                    for et in range(ET):
                        nc.tensor.matmul(
                            ps[:], kT[et][:, sk * P:(sk + 1) * P], qt[et][:],
                            start=(et == 0), stop=(et == ET - 1))
                    ep = eppool.tile([P, NQ], BF16, tag="ep")
                    nc.scalar.activation(ep[:], ps[:], AF.Exp, scale=INV_SCALE)
                    if idx == 0:
                        nc.vector.tensor_copy(acc[:], ep[:])
                    else:
                        nc.vector.tensor_add(acc[:], acc[:], ep[:])
                    if prev is not None:
                        emit_pv(*prev, first=(idx == 1), last=False)
                    prev = (ep, sk)
                emit_pv(*prev, first=False, last=True)

                pz = [psz.tile([P, 1], F32, tag="pz", name="pz")
                      for _ in range(2)]
                for j in range(2):
                    nc.tensor.matmul(pz[j][:], acc[:, j * P:(j + 1) * P],
                                     ones[:], start=True, stop=True)

                for j in range(2):
                    zi = finpool.tile([P, 1], F32, tag="zi", name="zi")
                    nc.vector.reciprocal(zi[:], pz[j][:])
                    ob = finpool.tile([P, E], F32, tag="ob", name="ob")
                    r = qc * 2 + j
                    # half-column copy/DMA pairs pipeline the output drain
                    # (ScalarE copy of half N overlaps the other half's DMA)
                    for oh in range(2):
                        cs = slice(oh * 512, (oh + 1) * 512)
                        nc.scalar.activation(ob[:, cs], po[j][:, cs],
                                             AF.Copy, scale=zi[:])
                        if with_bv:
                            nc.vector.tensor_add(ob[:, cs], ob[:, cs],
                                                 bv_sb[:, cs])
                        nc.gpsimd.dma_start(out[r * P:(r + 1) * P, cs],
                                            ob[:, cs])
        qxpool.release()
        wqpool.release()


def _build(with_bv):
    key = ("nc", with_bv)
    if key in _CACHE:
        return _CACHE[key]
    nc = bacc.Bacc("TRN2", target_bir_lowering=False, debug=False,
                   num_devices=N_CORES)
    dram = (
        nc.dram_tensor("xqT", [E, SQ], BF16, kind="ExternalInput"),
        nc.dram_tensor("xkT", [E, SQ], BF16, kind="ExternalInput"),
        nc.dram_tensor("xvT", [E, SQ], BF16, kind="ExternalInput"),
        nc.dram_tensor("wqT", [E, E], BF16, kind="ExternalInput"),
        nc.dram_tensor("wkT", [E, E], BF16, kind="ExternalInput"),
        nc.dram_tensor("wvT", [E, E], BF16, kind="ExternalInput"),
        nc.dram_tensor("bqr", [P, ET], F32, kind="ExternalInput"),
        nc.dram_tensor("bkr", [P, ET], F32, kind="ExternalInput"),
        nc.dram_tensor("bvb", [P, E], F32, kind="ExternalInput"),
        nc.dram_tensor("out", [SQ, E], F32, kind="ExternalOutput"),
    )
    with tile.TileContext(nc) as tc:
        _emit(nc, tc, dram, with_bv)
    nc.compile()
    _CACHE[key] = nc
    return nc


def _prep_in_maps(query, key, value, Wq, bq, Wk, bk, Wv, bv):
    bf = ml_dtypes.bfloat16
    wqT = np.ascontiguousarray(np.asarray(Wq, np.float32).T.astype(bf))
    wkT = np.ascontiguousarray(np.asarray(Wk, np.float32).T.astype(bf))
    wvT = np.ascontiguousarray(np.asarray(Wv, np.float32).T.astype(bf))
    bqr = np.ascontiguousarray(np.asarray(bq, np.float32).reshape(ET, P).T)
    bkr = np.ascontiguousarray(np.asarray(bk, np.float32).reshape(ET, P).T)
    bvb = np.ascontiguousarray(
        np.broadcast_to(np.asarray(bv, np.float32), (P, E)))
    query = np.asarray(query, np.float32)
    key = np.asarray(key, np.float32)
    value = np.asarray(value, np.float32)
    in_maps = []
    for c in range(N_CORES):
        b, h = divmod(c, HQ)
        sl = slice(h * SQ, (h + 1) * SQ)
        in_maps.append({
            "xqT": np.ascontiguousarray(query[b, sl, :].T.astype(bf)),
            "xkT": np.ascontiguousarray(key[b, sl, :].T.astype(bf)),
            "xvT": np.ascontiguousarray(value[b, sl, :].T.astype(bf)),
            "wqT": wqT, "wkT": wkT, "wvT": wvT,
            "bqr": bqr, "bkr": bkr, "bvb": bvb,
        })
    return in_maps


def kernel(query, key, value, Wq, bq, Wk, bk, Wv, bv, _run_kwargs=None):
    with_bv = bool(np.any(np.asarray(bv, np.float32)))
    nc = _build(with_bv)
    in_maps = _prep_in_maps(query, key, value, Wq, bq, Wk, bk, Wv, bv)
    res = run_bass_kernel_spmd(nc, in_maps, core_ids=list(range(N_CORES)),
                               **(_run_kwargs or {}))
    out = np.empty((B, S, E), np.float32)
    for c in range(N_CORES):
        b, h = divmod(c, HQ)
        out[b, h * SQ:(h + 1) * SQ, :] = res.results[c]["out"]
    if _run_kwargs:
        _CACHE["last_results"] = res
    return out



# revision 30
# speedup vs baseline: 1.0807x; 1.0807x over previous
"""Trainium2 Bass kernel: batched dense attention
   out = softmax((x_q Wq^T + bq)(x_k Wk^T + bk)^T / sqrt(E)) (x_v Wv^T + bv)

Sharding: 8 cores = 4 batches x 2 query-row halves (sequence-parallel on
Q). K/V projections are also split across the pair (each core projects
its own half of K/V) and the halves are exchanged with chunked in-pair
AllGathers that pipeline behind the projection matmuls, so every core
does exactly 1/8 of the total FLOPs.

Key trick: softmax attention is invariant to a permutation of the key
axis, so each core uses its own LOCAL key order [my half; partner half].
The locally projected half is written straight into the resident SBUF
K^T/V tiles (no DRAM round trip), and only the partner block of each
AllGather output is loaded back, addressed with a partition_id-derived
dynamic DRAM offset.

Device scheme (matmul operands bf16, fp32 PSUM accumulation):
  - scores are computed TRANSPOSED (keys on partitions) so the exp'd
    probabilities feed the P@V matmul as the stationary operand with no
    on-device transposes (host pre-transposes x^T / W^T and pre-casts
    to bf16).
  - softmax without max-subtraction (scores ~ N(0,1) at this scale;
    exp is safe in fp32): Z = sum_k exp(s) accumulated via a
    ones-vector matmul; 1/Z applied during the PSUM->SBUF output copy.
  - attention consumes key-tiles in availability order (own tiles
    first within each gather chunk), and the P@V matmuls trail the
    score/exp pipeline by one key-tile so the PSUM output-slot handoff
    between query chunks stays off the TensorE critical path.
  - bv folded to the end (softmax rows sum to 1 => P@(V+bv) = P@V+bv),
    and skipped entirely when bv == 0 (a per-bias-pattern kernel
    variant is compiled).
"""

import numpy as np
import ml_dtypes

import concourse.bacc as bacc
import concourse.bass as bass
import concourse.mybir as mybir
import concourse.tile as tile
from concourse.bass_utils import run_bass_kernel_spmd

B, S, E = 4, 4096, 1024
N_CORES = 8
HQ = 2              # halves per batch
SQ = S // HQ        # 2048 rows per core (own query rows / own K,V rows)
P = 128
ET = E // P         # 8 embed tiles
SKT = S // P        # 32 key tiles (local order: 0-15 own, 16-31 partner)
NQ = 256            # attention query-chunk (2 psum out subtiles)
NQT = SQ // NQ      # 8 chunks per core
CH = 512            # projection column chunk
HCH = SQ // CH      # 4 chunks for the K/V half projections
INV_SCALE = 1.0 / float(E) ** 0.5
GROUPS = [[0, 1], [2, 3], [4, 5], [6, 7]]

# key-tile processing order: chunk ch covers own tiles {4ch..4ch+3}
# (ready right after the projection) and partner tiles {16+4ch..}
# (ready after that chunk's AllGather)
SK_ORDER = list(range(16)) + list(range(16, 32))

BF16 = mybir.dt.bfloat16
F32 = mybir.dt.float32
AF = mybir.ActivationFunctionType

_CACHE = {}


def _emit(nc, tc, dram, with_bv):
    xqT, xkT, xvT, wqT, wkT, wvT, bqr, bkr, bvb, out = dram

    # partner-block row offsets into the AllGather outputs (per-engine
    # registers: dynamic DMA offsets must live on the issuing engine)
    pid_g = nc.gpsimd.partition_id()
    off_k = (1 - (pid_g & 1)) * E
    off_v = (1 - (pid_g & 1)) * CH

    with (
        tc.tile_pool(name="consts", bufs=1) as cpool,
        tc.tile_pool(name="kv", bufs=1) as kvpool,
        tc.tile_pool(name="ccd", bufs=1, space="DRAM") as dpool,
    ):
        bq_sb = cpool.tile([P, ET], F32)
        nc.gpsimd.dma_start(bq_sb[:], bqr[:])
        bk_sb = cpool.tile([P, ET], F32)
        nc.gpsimd.dma_start(bk_sb[:], bkr[:])
        if with_bv:
            bv_sb = cpool.tile([P, E], F32)
            nc.gpsimd.dma_start(bv_sb[:], bvb[:])

        # tiny warm-up collective: pays the first-collective comm setup
        # (~25us) before the real exchange needs the CC cores
        warm_in = dpool.tile([1, ET], F32, tag="warm_i", name="warm_i")
        warm_out = dpool.tile([HQ, ET], F32, tag="warm_o", name="warm_o")
        nc.gpsimd.dma_start(warm_in[:], bqr[0:1, :])
        nc.gpsimd.collective_compute(
            "AllGather", mybir.AluOpType.bypass, replica_groups=GROUPS,
            ins=[warm_in.opt()], outs=[warm_out.opt()])

        ones = cpool.tile([P, 1], F32)
        nc.gpsimd.memset(ones[:], 1.0)

        wqpool = tc.alloc_tile_pool(name="wq", bufs=1)
        wq_sb = [wqpool.tile([P, E], BF16, tag=f"wq{dt}", name=f"wq{dt}")
                 for dt in range(ET)]

        # resident full K^T [E, S] and V [S, E] (bf16, 16 MB), local key
        # order. Own and partner K^T halves are separate tiles so the
        # deferred partner-block DMAs never order against attention's reads
        # of the own half (dependencies track whole tiles).
        kTo = [kvpool.tile([P, SQ], BF16, tag=f"kTo{et}", name=f"kTo{et}")
               for et in range(ET)]
        kTp = [kvpool.tile([P, SQ], BF16, tag=f"kTp{et}", name=f"kTp{et}")
               for et in range(ET)]

        def kT(et, sk):
            t, lo = (kTo, sk) if sk < 16 else (kTp, sk - 16)
            return t[et][:, lo * P:(lo + 1) * P]

        vN = [kvpool.tile([P, E], BF16, tag=f"v{sk}", name=f"v{sk}")
              for sk in range(SKT)]

        # per-chunk AllGather bounce buffers
        kb_ib = [dpool.tile([E, CH], BF16, tag=f"kbi{c}", name=f"kbi{c}")
                 for c in range(HCH)]
        kb_ob = [dpool.tile([HQ * E, CH], BF16, tag=f"kbo{c}", name=f"kbo{c}")
                 for c in range(HCH)]
        vb_ib = [dpool.tile([CH, E], BF16, tag=f"vbi{c}", name=f"vbi{c}")
                 for c in range(HCH)]
        vb_ob = [dpool.tile([HQ * CH, E], BF16, tag=f"vbo{c}", name=f"vbo{c}")
                 for c in range(HCH)]

        # ---- Phase 1: half K / half V projections + pipelined AllGathers ----
        qxpool = tc.alloc_tile_pool(name="qx", bufs=8)
        xq0 = [qxpool.tile([P, NQ], BF16, tag="qx", name="xq0")
               for _ in range(ET)]
        with (
            tc.tile_pool(name="wkv", bufs=16) as wpool,
            tc.tile_pool(name="xk", bufs=14) as xkpool,
            tc.tile_pool(name="xv", bufs=13) as xvpool,
            tc.tile_pool(name="psk", bufs=3, space="PSUM") as psk,
            tc.tile_pool(name="psv", bufs=4, space="PSUM") as psv,
        ):
            wk_sb = [wpool.tile([P, E], BF16, tag="w", name="wk")
                     for _ in range(ET)]
            for dt in range(ET):
                nc.scalar.dma_start(wk_sb[dt][:], wkT[dt * P:(dt + 1) * P, :])

            def feed(src_t, ch, pool):
                cs = slice(ch * CH, (ch + 1) * CH)
                ts = [pool.tile([P, CH], BF16, tag="x", name="xf")
                      for _ in range(ET)]
                for dt in range(ET):
                    nc.sync.dma_start(ts[dt][:], src_t[dt * P:(dt + 1) * P, cs])
                return ts

            feeds = {("k", 0): feed(xkT, 0, xkpool),
                     ("v", 0): feed(xvT, 0, xvpool)}
            wv_sb = [wpool.tile([P, E], BF16, tag="w", name="wv")
                     for _ in range(ET)]
            for dt in range(ET):
                nc.scalar.dma_start(wv_sb[dt][:], wvT[dt * P:(dt + 1) * P, :])
            for dt in range(ET):
                nc.scalar.dma_start(wq_sb[dt][:], wqT[dt * P:(dt + 1) * P, :])
            for dt in range(ET):
                nc.sync.dma_start(xq0[dt][:], xqT[dt * P:(dt + 1) * P, 0:NQ])
            for ch in range(HCH):
                cs = slice(ch * CH, (ch + 1) * CH)
                # own-half k^T[e, s-chunk] = sum_d WkT[d, e] * xkT[d, s-chunk]
                # written straight into kT columns [0, SQ)
                xs = feeds.pop(("k", ch))
                if ch + 1 < HCH:
                    feeds[("k", ch + 1)] = feed(xkT, ch + 1, xkpool)
                for et in range(ET):
                    ps = psk.tile([P, CH], F32, tag="pk")
                    for dt in range(ET):
                        nc.tensor.matmul(
                            ps[:], wk_sb[dt][:, et * P:(et + 1) * P], xs[dt][:],
                            start=(dt == 0), stop=(dt == ET - 1))
                    nc.vector.tensor_scalar_add(
                        kTo[et][:, cs], ps[:], bk_sb[:, et:et + 1])
                    nc.gpsimd.dma_start(kb_ib[ch][et * P:(et + 1) * P, :],
                                        kTo[et][:, cs])
                nc.gpsimd.collective_compute(
                    "AllGather", mybir.AluOpType.bypass, replica_groups=GROUPS,
                    ins=[kb_ib[ch].opt()], outs=[kb_ob[ch].opt()])

                # own-half v[s-chunk, e] = sum_d xvT[d, s-chunk] * WvT[d, e]
                xv = feeds.pop(("v", ch))
                if ch + 1 < HCH:
                    feeds[("v", ch + 1)] = feed(xvT, ch + 1, xvpool)
                for si in range(CH // P):
                    sk = ch * (CH // P) + si
                    ph = [psv.tile([P, 512], F32, tag="pv", name="pv")
                          for _ in range(2)]
                    for dt in range(ET):
                        for nh in range(2):
                            nc.tensor.matmul(
                                ph[nh][:],
                                xv[dt][:, si * P:(si + 1) * P],
                                wv_sb[dt][:, nh * 512:(nh + 1) * 512],
                                start=(dt == 0), stop=(dt == ET - 1))
                    for nh in range(2):
                        nc.vector.tensor_copy(
                            vN[sk][:, nh * 512:(nh + 1) * 512], ph[nh][:])
                    nc.gpsimd.dma_start(vb_ib[ch][si * P:(si + 1) * P, :],
                                        vN[sk][:])
                nc.gpsimd.collective_compute(
                    "AllGather", mybir.AluOpType.bypass, replica_groups=GROUPS,
                    ins=[vb_ib[ch].opt()], outs=[vb_ob[ch].opt()])

            # partner blocks -> kT columns [SQ, S) / vN tiles 16..31, deferred
            # past the projection phase: DRAM there is bandwidth-saturated by
            # feeds + bounce traffic, while attention needs these tiles only
            # ~100us later (key tiles 16+ of the first query chunk)
            for ch in range(HCH):
                for et in range(ET):
                    nc.gpsimd.dma_start(
                        kTp[et][:, ch * CH:(ch + 1) * CH],
                        kb_ob[ch][bass.ds(off_k + et * P, P), :])
                for si in range(CH // P):
                    nc.gpsimd.dma_start(
                        vN[16 + ch * (CH // P) + si][:],
                        vb_ob[ch][bass.ds(off_v + si * P, P), :])

        # ---------------- Phase 2: attention ----------------
        with (
            tc.tile_pool(name="qt", bufs=16) as qtpool,
            tc.tile_pool(name="ep", bufs=4) as eppool,
            tc.tile_pool(name="acc", bufs=2) as accpool,
            tc.tile_pool(name="fin", bufs=4) as finpool,
            tc.tile_pool(name="pss", bufs=2, space="PSUM") as pss,
            tc.tile_pool(name="pso", bufs=2, space="PSUM") as pso,
            tc.tile_pool(name="psz", bufs=2, space="PSUM") as psz,
        ):
            for qc in range(NQT):
                # q^T[e, chunk] = sum_d WqT[d, e] * xqT[d, chunk]  (+ bq)
                # computed just-in-time so attention starts while the K/V
                # exchange tail is still in flight
                if qc == 0:
                    xq = xq0
                else:
                    xq = [qxpool.tile([P, NQ], BF16, tag="qx", name="xq")
                          for _ in range(ET)]
                    for dt in range(ET):
                        nc.sync.dma_start(
                            xq[dt][:],
                            xqT[dt * P:(dt + 1) * P, qc * NQ:(qc + 1) * NQ])
                qt = [qtpool.tile([P, NQ], BF16, tag="qt", name="qt")
                      for et in range(ET)]
                for et in range(ET):
                    ps = pss.tile([P, NQ], F32, tag="ps")
                    for dt in range(ET):
                        nc.tensor.matmul(
                            ps[:], wq_sb[dt][:, et * P:(et + 1) * P], xq[dt][:],
                            start=(dt == 0), stop=(dt == ET - 1))
                    nc.vector.tensor_scalar_add(qt[et][:], ps[:], bq_sb[:, et:et + 1])
                po = [pso.tile([P, E], F32, tag="po", name="po")
                      for _ in range(2)]

                def emit_pv(ep, sk, first, last):
                    for j in range(2):
                        lhs = ep[:, j * P:(j + 1) * P]
                        for nh in range(2):
                            nc.tensor.matmul(
                                po[j][:, nh * 512:(nh + 1) * 512], lhs,
                                vN[sk][:, nh * 512:(nh + 1) * 512],
                                start=first, stop=last)

                # scores/exp run one key-tile ahead of the P@V accumulation.
                # VectorE accumulates the exp'd tiles (acc = sum_sk ep) so the
                # softmax denominator needs only 2 one-column matmuls per qc
                # (partition-reduce of acc) instead of one per (key-tile, j).
                acc = accpool.tile([P, NQ], F32, tag="acc", name="acc")
                prev = None
                for idx, sk in enumerate(SK_ORDER):
                    ps = pss.tile([P, NQ], F32, tag="ps")
                    for et in range(ET):
                        nc.tensor.matmul(
                            ps[:], kT(et, sk), qt[et][:],
                            start=(et == 0), stop=(et == ET - 1))
                    ep = eppool.tile([P, NQ], BF16, tag="ep")
                    nc.scalar.activation(ep[:], ps[:], AF.Exp, scale=INV_SCALE)
                    if idx == 0:
                        nc.vector.tensor_copy(acc[:], ep[:])
                    else:
                        nc.vector.tensor_add(acc[:], acc[:], ep[:])
                    if prev is not None:
                        emit_pv(*prev, first=(idx == 1), last=False)
                    prev = (ep, sk)
                emit_pv(*prev, first=False, last=True)

                pz = [psz.tile([P, 1], F32, tag="pz", name="pz")
                      for _ in range(2)]
                for j in range(2):
                    nc.tensor.matmul(pz[j][:], acc[:, j * P:(j + 1) * P],
                                     ones[:], start=True, stop=True)

                for j in range(2):
                    zi = finpool.tile([P, 1], F32, tag="zi", name="zi")
                    nc.vector.reciprocal(zi[:], pz[j][:])
                    ob = finpool.tile([P, E], F32, tag="ob", name="ob")
                    r = qc * 2 + j
                    # half-column copy/DMA pairs pipeline the output drain
                    # (ScalarE copy of half N overlaps the other half's DMA)
                    for oh in range(2):
                        cs = slice(oh * 512, (oh + 1) * 512)
                        nc.scalar.activation(ob[:, cs], po[j][:, cs],
                                             AF.Copy, scale=zi[:])
                        if with_bv:
                            nc.vector.tensor_add(ob[:, cs], ob[:, cs],
                                                 bv_sb[:, cs])
                        nc.gpsimd.dma_start(out[r * P:(r + 1) * P, cs],
                                            ob[:, cs])
        qxpool.release()
        wqpool.release()


def _build(with_bv):
    key = ("nc", with_bv)
    if key in _CACHE:
        return _CACHE[key]
    nc = bacc.Bacc("TRN2", target_bir_lowering=False, debug=False,
                   num_devices=N_CORES)
    dram = (
        nc.dram_tensor("xqT", [E, SQ], BF16, kind="ExternalInput"),
        nc.dram_tensor("xkT", [E, SQ], BF16, kind="ExternalInput"),
        nc.dram_tensor("xvT", [E, SQ], BF16, kind="ExternalInput"),
        nc.dram_tensor("wqT", [E, E], BF16, kind="ExternalInput"),
        nc.dram_tensor("wkT", [E, E], BF16, kind="ExternalInput"),
        nc.dram_tensor("wvT", [E, E], BF16, kind="ExternalInput"),
        nc.dram_tensor("bqr", [P, ET], F32, kind="ExternalInput"),
        nc.dram_tensor("bkr", [P, ET], F32, kind="ExternalInput"),
        nc.dram_tensor("bvb", [P, E], F32, kind="ExternalInput"),
        nc.dram_tensor("out", [SQ, E], F32, kind="ExternalOutput"),
    )
    with tile.TileContext(nc) as tc:
        _emit(nc, tc, dram, with_bv)
    nc.compile()
    _CACHE[key] = nc
    return nc


def _prep_in_maps(query, key, value, Wq, bq, Wk, bk, Wv, bv):
    bf = ml_dtypes.bfloat16
    wqT = np.ascontiguousarray(np.asarray(Wq, np.float32).T.astype(bf))
    wkT = np.ascontiguousarray(np.asarray(Wk, np.float32).T.astype(bf))
    wvT = np.ascontiguousarray(np.asarray(Wv, np.float32).T.astype(bf))
    bqr = np.ascontiguousarray(np.asarray(bq, np.float32).reshape(ET, P).T)
    bkr = np.ascontiguousarray(np.asarray(bk, np.float32).reshape(ET, P).T)
    bvb = np.ascontiguousarray(
        np.broadcast_to(np.asarray(bv, np.float32), (P, E)))
    query = np.asarray(query, np.float32)
    key = np.asarray(key, np.float32)
    value = np.asarray(value, np.float32)
    in_maps = []
    for c in range(N_CORES):
        b, h = divmod(c, HQ)
        sl = slice(h * SQ, (h + 1) * SQ)
        in_maps.append({
            "xqT": np.ascontiguousarray(query[b, sl, :].T.astype(bf)),
            "xkT": np.ascontiguousarray(key[b, sl, :].T.astype(bf)),
            "xvT": np.ascontiguousarray(value[b, sl, :].T.astype(bf)),
            "wqT": wqT, "wkT": wkT, "wvT": wvT,
            "bqr": bqr, "bkr": bkr, "bvb": bvb,
        })
    return in_maps


def kernel(query, key, value, Wq, bq, Wk, bk, Wv, bv, _run_kwargs=None):
    with_bv = bool(np.any(np.asarray(bv, np.float32)))
    nc = _build(with_bv)
    in_maps = _prep_in_maps(query, key, value, Wq, bq, Wk, bk, Wv, bv)
    res = run_bass_kernel_spmd(nc, in_maps, core_ids=list(range(N_CORES)),
                               **(_run_kwargs or {}))
    out = np.empty((B, S, E), np.float32)
    for c in range(N_CORES):
        b, h = divmod(c, HQ)
        out[b, h * SQ:(h + 1) * SQ, :] = res.results[c]["out"]
    if _run_kwargs:
        _CACHE["last_results"] = res
    return out



# revision 32
# speedup vs baseline: 1.0882x; 1.0069x over previous
"""Trainium2 Bass kernel: batched dense attention
   out = softmax((x_q Wq^T + bq)(x_k Wk^T + bk)^T / sqrt(E)) (x_v Wv^T + bv)

Sharding: 8 cores = 4 batches x 2 query-row halves (sequence-parallel on
Q). K/V projections are also split across the pair (each core projects
its own half of K/V) and the halves are exchanged with chunked in-pair
AllGathers that pipeline behind the projection matmuls, so every core
does exactly 1/8 of the total FLOPs.

Key trick: softmax attention is invariant to a permutation of the key
axis, so each core uses its own LOCAL key order [my half; partner half].
The locally projected half is written straight into the resident SBUF
K^T/V tiles (no DRAM round trip), and only the partner block of each
AllGather output is loaded back, addressed with a partition_id-derived
dynamic DRAM offset.

Device scheme (matmul operands bf16, fp32 PSUM accumulation):
  - scores are computed TRANSPOSED (keys on partitions) so the exp'd
    probabilities feed the P@V matmul as the stationary operand with no
    on-device transposes (host pre-transposes x^T / W^T and pre-casts
    to bf16).
  - softmax without max-subtraction (scores ~ N(0,1) at this scale;
    exp is safe in fp32): Z = sum_k exp(s) accumulated via a
    ones-vector matmul; 1/Z applied during the PSUM->SBUF output copy.
  - attention consumes key-tiles in availability order (own tiles
    first within each gather chunk), and the P@V matmuls trail the
    score/exp pipeline by one key-tile so the PSUM output-slot handoff
    between query chunks stays off the TensorE critical path.
  - bv folded to the end (softmax rows sum to 1 => P@(V+bv) = P@V+bv),
    and skipped entirely when bv == 0 (a per-bias-pattern kernel
    variant is compiled).
"""

import numpy as np
import ml_dtypes

import concourse.bacc as bacc
import concourse.bass as bass
import concourse.mybir as mybir
import concourse.tile as tile
from concourse.bass_utils import run_bass_kernel_spmd

B, S, E = 4, 4096, 1024
N_CORES = 8
HQ = 2              # halves per batch
SQ = S // HQ        # 2048 rows per core (own query rows / own K,V rows)
P = 128
ET = E // P         # 8 embed tiles
SKT = S // P        # 32 key tiles (local order: 0-15 own, 16-31 partner)
NQ = 256            # attention query-chunk (2 psum out subtiles)
NQT = SQ // NQ      # 8 chunks per core
CH = 512            # projection column chunk
HCH = SQ // CH      # 4 chunks for the K/V half projections
INV_SCALE = 1.0 / float(E) ** 0.5
GROUPS = [[0, 1], [2, 3], [4, 5], [6, 7]]

# key-tile processing order: chunk ch covers own tiles {4ch..4ch+3}
# (ready right after the projection) and partner tiles {16+4ch..}
# (ready after that chunk's AllGather)
SK_ORDER = list(range(16)) + list(range(16, 32))

BF16 = mybir.dt.bfloat16
F32 = mybir.dt.float32
AF = mybir.ActivationFunctionType

_CACHE = {}


def _emit(nc, tc, dram, with_bv):
    xqT, xkT, xvT, wqT, wkT, wvT, bqr, bkr, bvb, out = dram

    # partner-block row offsets into the AllGather outputs (the register
    # lives on ScalarE, which issues the partner-load DMAs)
    pid = nc.scalar.partition_id()
    off_k = (1 - (pid & 1)) * E
    off_v = (1 - (pid & 1)) * CH

    with (
        tc.tile_pool(name="consts", bufs=1) as cpool,
        tc.tile_pool(name="kv", bufs=1) as kvpool,
        tc.tile_pool(name="ccd", bufs=1, space="DRAM") as dpool,
    ):
        bq_sb = cpool.tile([P, ET], F32)
        nc.gpsimd.dma_start(bq_sb[:], bqr[:])
        bk_sb = cpool.tile([P, ET], F32)
        nc.gpsimd.dma_start(bk_sb[:], bkr[:])
        if with_bv:
            bv_sb = cpool.tile([P, E], F32)
            nc.gpsimd.dma_start(bv_sb[:], bvb[:])

        # tiny warm-up collective: pays the first-collective comm setup
        # (~25us) before the real exchange needs the CC cores
        warm_in = dpool.tile([1, ET], F32, tag="warm_i", name="warm_i")
        warm_out = dpool.tile([HQ, ET], F32, tag="warm_o", name="warm_o")
        nc.gpsimd.dma_start(warm_in[:], bqr[0:1, :])
        nc.gpsimd.collective_compute(
            "AllGather", mybir.AluOpType.bypass, replica_groups=GROUPS,
            ins=[warm_in.opt()], outs=[warm_out.opt()])

        ones = cpool.tile([P, 1], F32)
        nc.gpsimd.memset(ones[:], 1.0)

        wqpool = tc.alloc_tile_pool(name="wq", bufs=1)
        wq_sb = [wqpool.tile([P, E], BF16, tag=f"wq{dt}", name=f"wq{dt}")
                 for dt in range(ET)]

        # resident full K^T [E, S] and V [S, E] (bf16, 16 MB), local key
        # order. Own and partner K^T halves are separate tiles so the
        # deferred partner-block DMAs never order against attention's reads
        # of the own half (dependencies track whole tiles).
        kTo = [kvpool.tile([P, SQ], BF16, tag=f"kTo{et}", name=f"kTo{et}")
               for et in range(ET)]
        kTp = [kvpool.tile([P, SQ], BF16, tag=f"kTp{et}", name=f"kTp{et}")
               for et in range(ET)]

        def kT(et, sk):
            t, lo = (kTo, sk) if sk < 16 else (kTp, sk - 16)
            return t[et][:, lo * P:(lo + 1) * P]

        vN = [kvpool.tile([P, E], BF16, tag=f"v{sk}", name=f"v{sk}")
              for sk in range(SKT)]

        # per-chunk AllGather bounce buffers
        kb_ib = [dpool.tile([E, CH], BF16, tag=f"kbi{c}", name=f"kbi{c}")
                 for c in range(HCH)]
        kb_ob = [dpool.tile([HQ * E, CH], BF16, tag=f"kbo{c}", name=f"kbo{c}")
                 for c in range(HCH)]
        vb_ib = [dpool.tile([CH, E], BF16, tag=f"vbi{c}", name=f"vbi{c}")
                 for c in range(HCH)]
        vb_ob = [dpool.tile([HQ * CH, E], BF16, tag=f"vbo{c}", name=f"vbo{c}")
                 for c in range(HCH)]

        # ---- Phase 1: half K / half V projections + pipelined AllGathers ----
        qxpool = tc.alloc_tile_pool(name="qx", bufs=8)
        xq0 = [qxpool.tile([P, NQ], BF16, tag="qx", name="xq0")
               for _ in range(ET)]
        with (
            tc.tile_pool(name="wkv", bufs=16) as wpool,
            tc.tile_pool(name="xk", bufs=14) as xkpool,
            tc.tile_pool(name="xv", bufs=13) as xvpool,
            tc.tile_pool(name="psk", bufs=3, space="PSUM") as psk,
            tc.tile_pool(name="psv", bufs=4, space="PSUM") as psv,
        ):
            wk_sb = [wpool.tile([P, E], BF16, tag="w", name="wk")
                     for _ in range(ET)]
            for dt in range(ET):
                nc.scalar.dma_start(wk_sb[dt][:], wkT[dt * P:(dt + 1) * P, :])

            def feed(src_t, ch, pool):
                cs = slice(ch * CH, (ch + 1) * CH)
                ts = [pool.tile([P, CH], BF16, tag="x", name="xf")
                      for _ in range(ET)]
                for dt in range(ET):
                    nc.sync.dma_start(ts[dt][:], src_t[dt * P:(dt + 1) * P, cs])
                return ts

            feeds = {("k", 0): feed(xkT, 0, xkpool),
                     ("v", 0): feed(xvT, 0, xvpool)}
            wv_sb = [wpool.tile([P, E], BF16, tag="w", name="wv")
                     for _ in range(ET)]
            for dt in range(ET):
                nc.scalar.dma_start(wv_sb[dt][:], wvT[dt * P:(dt + 1) * P, :])
            for dt in range(ET):
                nc.scalar.dma_start(wq_sb[dt][:], wqT[dt * P:(dt + 1) * P, :])
            for dt in range(ET):
                nc.sync.dma_start(xq0[dt][:], xqT[dt * P:(dt + 1) * P, 0:NQ])
            for ch in range(HCH):
                cs = slice(ch * CH, (ch + 1) * CH)
                # own-half k^T[e, s-chunk] = sum_d WkT[d, e] * xkT[d, s-chunk]
                # written straight into kT columns [0, SQ)
                xs = feeds.pop(("k", ch))
                if ch + 1 < HCH:
                    feeds[("k", ch + 1)] = feed(xkT, ch + 1, xkpool)
                for et in range(ET):
                    ps = psk.tile([P, CH], F32, tag="pk")
                    for dt in range(ET):
                        nc.tensor.matmul(
                            ps[:], wk_sb[dt][:, et * P:(et + 1) * P], xs[dt][:],
                            start=(dt == 0), stop=(dt == ET - 1))
                    nc.vector.tensor_scalar_add(
                        kTo[et][:, cs], ps[:], bk_sb[:, et:et + 1])
                    nc.gpsimd.dma_start(kb_ib[ch][et * P:(et + 1) * P, :],
                                        kTo[et][:, cs])
                nc.gpsimd.collective_compute(
                    "AllGather", mybir.AluOpType.bypass, replica_groups=GROUPS,
                    ins=[kb_ib[ch].opt()], outs=[kb_ob[ch].opt()])

                # own-half v[s-chunk, e] = sum_d xvT[d, s-chunk] * WvT[d, e]
                xv = feeds.pop(("v", ch))
                if ch + 1 < HCH:
                    feeds[("v", ch + 1)] = feed(xvT, ch + 1, xvpool)
                for si in range(CH // P):
                    sk = ch * (CH // P) + si
                    ph = [psv.tile([P, 512], F32, tag="pv", name="pv")
                          for _ in range(2)]
                    for dt in range(ET):
                        for nh in range(2):
                            nc.tensor.matmul(
                                ph[nh][:],
                                xv[dt][:, si * P:(si + 1) * P],
                                wv_sb[dt][:, nh * 512:(nh + 1) * 512],
                                start=(dt == 0), stop=(dt == ET - 1))
                    for nh in range(2):
                        nc.vector.tensor_copy(
                            vN[sk][:, nh * 512:(nh + 1) * 512], ph[nh][:])
                    nc.gpsimd.dma_start(vb_ib[ch][si * P:(si + 1) * P, :],
                                        vN[sk][:])
                nc.gpsimd.collective_compute(
                    "AllGather", mybir.AluOpType.bypass, replica_groups=GROUPS,
                    ins=[vb_ib[ch].opt()], outs=[vb_ob[ch].opt()])

            # partner blocks -> kTp columns / vN tiles 16..31. Issued after
            # the whole projection loop on the otherwise-empty ScalarE queue:
            # each load is gated only on its own AllGather, so the transfers
            # trickle in as the gathers land without head-of-line blocking
            # the feeds (sync) or bounce copies (gpsimd).
            for ch in range(HCH):
                for et in range(ET):
                    nc.scalar.dma_start(
                        kTp[et][:, ch * CH:(ch + 1) * CH],
                        kb_ob[ch][bass.ds(off_k + et * P, P), :])
                for si in range(CH // P):
                    nc.scalar.dma_start(
                        vN[16 + ch * (CH // P) + si][:],
                        vb_ob[ch][bass.ds(off_v + si * P, P), :])

        # ---------------- Phase 2: attention ----------------
        with (
            tc.tile_pool(name="qt", bufs=16) as qtpool,
            tc.tile_pool(name="ep", bufs=4) as eppool,
            tc.tile_pool(name="acc", bufs=2) as accpool,
            tc.tile_pool(name="fin", bufs=4) as finpool,
            tc.tile_pool(name="pss", bufs=2, space="PSUM") as pss,
            tc.tile_pool(name="pso", bufs=2, space="PSUM") as pso,
            tc.tile_pool(name="psz", bufs=2, space="PSUM") as psz,
        ):
            for qc in range(NQT):
                # q^T[e, chunk] = sum_d WqT[d, e] * xqT[d, chunk]  (+ bq)
                # computed just-in-time so attention starts while the K/V
                # exchange tail is still in flight
                if qc == 0:
                    xq = xq0
                else:
                    xq = [qxpool.tile([P, NQ], BF16, tag="qx", name="xq")
                          for _ in range(ET)]
                    for dt in range(ET):
                        nc.sync.dma_start(
                            xq[dt][:],
                            xqT[dt * P:(dt + 1) * P, qc * NQ:(qc + 1) * NQ])
                qt = [qtpool.tile([P, NQ], BF16, tag="qt", name="qt")
                      for et in range(ET)]
                for et in range(ET):
                    ps = pss.tile([P, NQ], F32, tag="ps")
                    for dt in range(ET):
                        nc.tensor.matmul(
                            ps[:], wq_sb[dt][:, et * P:(et + 1) * P], xq[dt][:],
                            start=(dt == 0), stop=(dt == ET - 1))
                    nc.vector.tensor_scalar_add(qt[et][:], ps[:], bq_sb[:, et:et + 1])
                po = [pso.tile([P, E], F32, tag="po", name="po")
                      for _ in range(2)]

                def emit_pv(ep, sk, first, last):
                    for j in range(2):
                        lhs = ep[:, j * P:(j + 1) * P]
                        for nh in range(2):
                            nc.tensor.matmul(
                                po[j][:, nh * 512:(nh + 1) * 512], lhs,
                                vN[sk][:, nh * 512:(nh + 1) * 512],
                                start=first, stop=last)

                # scores/exp run one key-tile ahead of the P@V accumulation.
                # VectorE accumulates the exp'd tiles (acc = sum_sk ep) so the
                # softmax denominator needs only 2 one-column matmuls per qc
                # (partition-reduce of acc) instead of one per (key-tile, j).
                acc = accpool.tile([P, NQ], F32, tag="acc", name="acc")
                prev = None
                for idx, sk in enumerate(SK_ORDER):
                    ps = pss.tile([P, NQ], F32, tag="ps")
                    for et in range(ET):
                        nc.tensor.matmul(
                            ps[:], kT(et, sk), qt[et][:],
                            start=(et == 0), stop=(et == ET - 1))
                    ep = eppool.tile([P, NQ], BF16, tag="ep")
                    nc.scalar.activation(ep[:], ps[:], AF.Exp, scale=INV_SCALE)
                    if idx == 0:
                        nc.vector.tensor_copy(acc[:], ep[:])
                    else:
                        nc.vector.tensor_add(acc[:], acc[:], ep[:])
                    if prev is not None:
                        emit_pv(*prev, first=(idx == 1), last=False)
                    prev = (ep, sk)
                emit_pv(*prev, first=False, last=True)

                pz = [psz.tile([P, 1], F32, tag="pz", name="pz")
                      for _ in range(2)]
                for j in range(2):
                    nc.tensor.matmul(pz[j][:], acc[:, j * P:(j + 1) * P],
                                     ones[:], start=True, stop=True)

                for j in range(2):
                    zi = finpool.tile([P, 1], F32, tag="zi", name="zi")
                    nc.vector.reciprocal(zi[:], pz[j][:])
                    ob = finpool.tile([P, E], F32, tag="ob", name="ob")
                    r = qc * 2 + j
                    # half-column copy/DMA pairs pipeline the output drain
                    # (ScalarE copy of half N overlaps the other half's DMA)
                    for oh in range(2):
                        cs = slice(oh * 512, (oh + 1) * 512)
                        nc.scalar.activation(ob[:, cs], po[j][:, cs],
                                             AF.Copy, scale=zi[:])
                        if with_bv:
                            nc.vector.tensor_add(ob[:, cs], ob[:, cs],
                                                 bv_sb[:, cs])
                        nc.gpsimd.dma_start(out[r * P:(r + 1) * P, cs],
                                            ob[:, cs])
        qxpool.release()
        wqpool.release()


def _build(with_bv):
    key = ("nc", with_bv)
    if key in _CACHE:
        return _CACHE[key]
    nc = bacc.Bacc("TRN2", target_bir_lowering=False, debug=False,
                   num_devices=N_CORES)
    dram = (
        nc.dram_tensor("xqT", [E, SQ], BF16, kind="ExternalInput"),
        nc.dram_tensor("xkT", [E, SQ], BF16, kind="ExternalInput"),
        nc.dram_tensor("xvT", [E, SQ], BF16, kind="ExternalInput"),
        nc.dram_tensor("wqT", [E, E], BF16, kind="ExternalInput"),
        nc.dram_tensor("wkT", [E, E], BF16, kind="ExternalInput"),
        nc.dram_tensor("wvT", [E, E], BF16, kind="ExternalInput"),
        nc.dram_tensor("bqr", [P, ET], F32, kind="ExternalInput"),
        nc.dram_tensor("bkr", [P, ET], F32, kind="ExternalInput"),
        nc.dram_tensor("bvb", [P, E], F32, kind="ExternalInput"),
        nc.dram_tensor("out", [SQ, E], F32, kind="ExternalOutput"),
    )
    with tile.TileContext(nc) as tc:
        _emit(nc, tc, dram, with_bv)
    nc.compile()
    _CACHE[key] = nc
    return nc


def _prep_in_maps(query, key, value, Wq, bq, Wk, bk, Wv, bv):
    bf = ml_dtypes.bfloat16
    wqT = np.ascontiguousarray(np.asarray(Wq, np.float32).T.astype(bf))
    wkT = np.ascontiguousarray(np.asarray(Wk, np.float32).T.astype(bf))
    wvT = np.ascontiguousarray(np.asarray(Wv, np.float32).T.astype(bf))
    bqr = np.ascontiguousarray(np.asarray(bq, np.float32).reshape(ET, P).T)
    bkr = np.ascontiguousarray(np.asarray(bk, np.float32).reshape(ET, P).T)
    bvb = np.ascontiguousarray(
        np.broadcast_to(np.asarray(bv, np.float32), (P, E)))
    query = np.asarray(query, np.float32)
    key = np.asarray(key, np.float32)
    value = np.asarray(value, np.float32)
    in_maps = []
    for c in range(N_CORES):
        b, h = divmod(c, HQ)
        sl = slice(h * SQ, (h + 1) * SQ)
        in_maps.append({
            "xqT": np.ascontiguousarray(query[b, sl, :].T.astype(bf)),
            "xkT": np.ascontiguousarray(key[b, sl, :].T.astype(bf)),
            "xvT": np.ascontiguousarray(value[b, sl, :].T.astype(bf)),
            "wqT": wqT, "wkT": wkT, "wvT": wvT,
            "bqr": bqr, "bkr": bkr, "bvb": bvb,
        })
    return in_maps


def kernel(query, key, value, Wq, bq, Wk, bk, Wv, bv, _run_kwargs=None):
    with_bv = bool(np.any(np.asarray(bv, np.float32)))
    nc = _build(with_bv)
    in_maps = _prep_in_maps(query, key, value, Wq, bq, Wk, bk, Wv, bv)
    res = run_bass_kernel_spmd(nc, in_maps, core_ids=list(range(N_CORES)),
                               **(_run_kwargs or {}))
    out = np.empty((B, S, E), np.float32)
    for c in range(N_CORES):
        b, h = divmod(c, HQ)
        out[b, h * SQ:(h + 1) * SQ, :] = res.results[c]["out"]
    if _run_kwargs:
        _CACHE["last_results"] = res
    return out

